# revision 33
# baseline (speedup 1.0000x reference)
"""Trainium2 Bass kernel: single-head causal attention (B=16, T=2048, C=1024, HD=64).

Data-parallel over batch across 8 NeuronCores (2 batches/core), weights
replicated. Each core computes, per batch:
    q = x @ Wq, k = x @ Wk, v = x @ Wv            (via transposed layouts)
    scores[t, s] = k[t] . q[s] / sqrt(C)          (computed transposed: St[s, t])
    causal mask (keep s <= t), softmax over s, out[t] = sum_s w[t, s] v[s]

Pipeline layout (all matmuls bf16, PSUM fp32, softmax division fp32):
  - x tiles [128t, 1024c] are cast to bf16 (Pool/DVE), PE-transposed in bf16
    (full-rate loads + streams), and copied out of PSUM at DVE 2x-mode rate.
  - qT/kT come from one stacked [Wq|Wk] projection. Two swapped-stack SBUF
    tensors (qk = q@0:64/k@64:128, kq = k@0:64/q@64:128) let the K=64 score
    matmuls run two-at-a-time in separate PE row groups while needing only
    two cheap bf16 4x SBUF dup-copies per chunk.
  - v is projected with even/odd c-chunks in separate PE column groups,
    cast to bf16, PE-transposed; the two column-group halves are summed by
    a single small DVE add into V_aug [128s, 65] whose column 64 is 1.0 -
    the AV matmul then computes the numerator (rows 0:64) and the softmax
    denominator (row 64) at once.
  - St tile [128s, 512t] = matmul(lhsT=q slice, rhs=k chunk); ACT does
    exp(St/32) straight out of PSUM (no max-subtraction: scores have std
    1/4); gpsimd affine_select masks diagonal tiles; AV accumulates
    ut [65, 512t] in PSUM.
  - ut rows 0:65 are PE-transposed back to [t, 65]; one strided DVE
    reciprocal per chunk + ACT scale-copies produce the output tiles.
  - Emission interleaves projection work (PE/DVE/Pool-heavy) into the
    attention chunks (ACT-heavy) at single-matmul granularity so every
    engine stays dense and the PE holds its top p-state.
"""

import numpy as np

import concourse.bass as bass
import concourse.tile as tile
from concourse import bacc, mybir
from concourse.bass_utils import run_bass_kernel_spmd
from concourse.masks import make_identity

F32 = mybir.dt.float32
BF16 = mybir.dt.bfloat16
FP8 = mybir.dt.float8e4

B, T, C, HD = 16, 2048, 1024, 64
N_CORES = 8
BL = B // N_CORES  # batches per core

P = 128
TCH = 512  # t-chunk (PSUM bank width in fp32)


def build_attention(ctx, tc, out, x, wk, wq, wv, b_l, t_dim, c_dim):
    nc = tc.nc
    ncc = c_dim // P        # c chunks (contraction)
    nj = t_dim // TCH       # t chunks
    ntt = TCH // P          # t subtiles per chunk
    nst = t_dim // P        # s tiles
    scale = 1.0 / float(np.sqrt(np.float32(c_dim)))

    const_pool = ctx.enter_context(tc.tile_pool(name="const", bufs=1))
    io_pool = ctx.enter_context(tc.tile_pool(name="io", bufs=1))
    big_pool = ctx.enter_context(tc.tile_pool(name="big", bufs=1))
    per_b = ctx.enter_context(tc.tile_pool(name="per_b", bufs=1))
    work = ctx.enter_context(tc.tile_pool(name="work", bufs=1))
    ps = ctx.enter_context(tc.tile_pool(name="ps", bufs=1, space="PSUM"))

    # Identities arrive by DMA (gpsimd is busy generating x-load descriptors
    # at startup and the first PE transposes need identb immediately)
    eye_bf = (np.eye(P) * 0x3F80).astype(np.uint16)  # bf16 bits of 1.0
    identb_dram = nc.inline_tensor(eye_bf, name="identb_const")
    identb = const_pool.tile([P, P], BF16, name="identb")
    nc.sync.dma_start(identb, identb_dram.ap().bitcast(BF16))
    identf_dram = nc.inline_tensor(np.eye(P, dtype=np.float32), name="identf_const")
    identf = const_pool.tile([P, P], F32, name="identf")
    nc.sync.dma_start(identf, identf_dram.ap())

    # Stationary weights, cast to bf16: [c_inner=128, c_chunk, heads].
    wqk_f = const_pool.tile([P, ncc, 2 * HD], F32, name="wqk_f")
    nc.sync.dma_start(wqk_f[:, :, 0:HD], wq.rearrange("(o p) h -> p o h", p=P))
    nc.sync.dma_start(wqk_f[:, :, HD : 2 * HD], wk.rearrange("(o p) h -> p o h", p=P))
    wv_f = const_pool.tile([P, ncc, HD], F32, name="wv_f")
    nc.sync.dma_start(wv_f[:], wv.rearrange("(o p) h -> p o h", p=P))
    wqk_sb = const_pool.tile([P, ncc, 2 * HD], BF16, name="wqk_sb")
    nc.vector.tensor_copy(wqk_sb[:], wqk_f[:])
    wv_sb = const_pool.tile([P, ncc, HD], BF16, name="wv_sb")
    nc.vector.tensor_copy(wv_sb[:], wv_f[:])

    # q/k live in fp8 DoubleRow layout [32, 2, t]: head h of q^T sits at
    # partition h%32, second free index h//32 - the score matmuls contract
    # h as 32 partitions x 2 double-rows at half a cycle per column.
    # vaug holds V^T per s-tile with column 64 set to 1.0.
    qdr = {}
    kdr = {}
    vaug = {}
    for b in range(b_l):
        qdr[b] = per_b.tile([HD // 2, 2, t_dim], FP8, name=f"qdr_{b}", tag="qdr", bufs=2)
        kdr[b] = per_b.tile([HD // 2, 2, t_dim], FP8, name=f"kdr_{b}", tag="kdr", bufs=2)
        vaug[b] = per_b.tile(
            [P, nst, HD + 1], BF16, name=f"vaug_{b}", tag="vaug", bufs=2
        )
        nc.vector.memset(vaug[b][:, :, HD], 1.0)

    xins = {}

    def dma_chunk_ops(b, j, split=False):
        """x loads for chunk (b, j) as closures: converting DMAs (fp32 HBM ->
        bf16 SBUF) initiated by gpsimd (only the software DGE can cast).
        split=True halves each load along c so the first transposes (which
        need the low c-chunks of every tile) can start sooner."""
        tiles = []
        opsl = []
        for tt in range(ntt):
            xin = io_pool.tile(
                [P, c_dim], BF16, tag="xin", bufs=12, name=f"xin_{b}_{j}_{tt}"
            )
            tiles.append(xin)
        xins[(b, j)] = tiles

        def load(tt, lo, hi):
            def f():
                t0 = j * TCH + tt * P
                nc.gpsimd.dma_start(tiles[tt][:, lo:hi], x[b, t0 : t0 + P, lo:hi])

            return f

        if split and c_dim >= 2 * P:
            h = c_dim // 2
            for tt in range(ntt):
                opsl.append(load(tt, 0, h))
            for tt in range(ntt):
                opsl.append(load(tt, h, c_dim))
        else:
            for tt in range(ntt):
                opsl.append(load(tt, 0, c_dim))
        return opsl

    def a_chunk_ops(b, j):
        """Projection chunk: transpose, project q/k/v. Returns closures."""
        opsl = []
        xinb = xins.pop((b, j))

        xT = big_pool.tile(
            [P, ncc, TCH], BF16, tag="xT", bufs=2, name=f"xT_{b}_{j}"
        )
        tps = {}

        def tp_fill(g):
            def f():
                tp = ps.tile(
                    [P, 2, TCH], BF16, tag="tp", bufs=3, name=f"tp_{b}_{j}_{g}"
                )
                tps[g] = tp
                for ccp in range(2):
                    cc = 2 * g + ccp
                    for tt in range(ntt):
                        nc.tensor.transpose(
                            tp[:, ccp, tt * P : (tt + 1) * P],
                            xinb[tt][:, cc * P : (cc + 1) * P],
                            identb,
                        )

            return f

        def tp_copy(g):
            def f():
                nc.vector.tensor_copy(xT[:, 2 * g : 2 * g + 2, :], tps.pop(g))

            return f

        for g in range(ncc // 2):
            opsl.append(tp_fill(g))
            opsl.append(tp_copy(g))

        qkps_box = {}

        def qk_mm(c0, c1):
            def f():
                if "t" not in qkps_box:
                    qkps_box["t"] = ps.tile(
                        [P, TCH], F32, tag="tp", bufs=3, name=f"qkps_{b}_{j}"
                    )
                for cc in range(c0, c1):
                    nc.tensor.matmul(
                        qkps_box["t"],
                        wqk_sb[:, cc, :],
                        xT[:, cc, :],
                        start=(cc == 0),
                        stop=(cc == ncc - 1),
                        skip_group_check=True,
                    )

            return f

        opsl.append(qk_mm(0, ncc // 2))
        opsl.append(qk_mm(ncc // 2, ncc))

        jt = slice(j * TCH, (j + 1) * TCH)

        def qk_out():
            # fold q (psum rows 0:64) and k (rows 64:128) into the fp8
            # DoubleRow layout; split the casts across DVE and ACT
            qkps = qkps_box["t"]
            h2 = HD // 2
            nc.vector.tensor_copy(qdr[b][:, 0, jt], qkps[0:h2, :])
            nc.scalar.copy(qdr[b][:, 1, jt], qkps[h2:HD, :])
            nc.vector.tensor_copy(kdr[b][:, 0, jt], qkps[HD : HD + h2, :])
            nc.scalar.copy(kdr[b][:, 1, jt], qkps[HD + h2 : P, :])

        opsl.append(qk_out)

        vps_box = {}

        def v_mm(c0, c1):
            def f():
                if "t" not in vps_box:
                    vps_box["t"] = ps.tile(
                        [P, TCH], F32, tag="tp", bufs=3, name=f"vps_{b}_{j}"
                    )
                vps = vps_box["t"]
                for cc in range(c0, c1):
                    h = cc % 2
                    nc.tensor.matmul(
                        vps[h * HD : (h + 1) * HD, :],
                        wv_sb[:, cc, :],
                        xT[:, cc, :],
                        start=(cc == h),
                        stop=(cc == ncc - 2 + h),
                        tile_position=(0, h * HD),
                        skip_group_check=True,
                    )

            return f

        opsl.append(v_mm(0, ncc // 2))
        opsl.append(v_mm(ncc // 2, ncc))

        vsb_box = {}

        def v_out():
            vsb_box["t"] = work.tile([P, TCH], BF16, tag="vsb", bufs=2, name=f"vs_{b}_{j}")
            nc.vector.tensor_copy(vsb_box["t"], vps_box["t"])  # cast

        opsl.append(v_out)

        vtp_box = {}

        def v_tp():
            vtp = ps.tile([P, 2, TCH], BF16, tag="tp", bufs=3, name=f"vtp_{b}_{j}")
            vtp_box["t"] = vtp
            vsb = vsb_box["t"]
            for tt in range(ntt):
                nc.tensor.transpose(
                    vtp[:, 0, tt * P : (tt + 1) * P],
                    vsb[:, tt * P : (tt + 1) * P],
                    identb,
                )

        opsl.append(v_tp)

        def v_aug():
            vtsb = work.tile([P, ntt, P], BF16, tag="vtsb", bufs=2, name=f"vt_{b}_{j}")
            nc.vector.tensor_copy(vtsb, vtp_box["t"][:, 0, :])
            # sum the two column-group halves of every subtile in one op
            nc.vector.tensor_add(
                vaug[b][:, j * ntt : (j + 1) * ntt, 0:HD],
                vtsb[:, :, 0:HD],
                vtsb[:, :, HD:P],
            )

        opsl.append(v_aug)
        return opsl

    def b_chunk_ops(b, j):
        """Attention chunk: scores, exp, mask, AV, softmax-divide, store."""
        opsl = []
        ni = ntt * j + ntt  # s-tiles with any valid (s <= t) entry
        jt = slice(j * TCH, (j + 1) * TCH)
        pts = {}
        box = {}

        def s_half(stq, hh, i):
            """Scores for s-tile i into stq half hh: one fp8 DoubleRow matmul
            (K=64 as 32 partitions x 2 rows, half a cycle per column)."""
            nc.tensor.matmul(
                stq[:, hh, :],
                qdr[b][:, :, i * P : (i + 1) * P],
                kdr[b][:, :, jt],
                start=True,
                stop=True,
                perf_mode=mybir.MatmulPerfMode.DoubleRow,
                skip_group_check=True,
            )

        def s_exp(p):
            """Scores + exp for the s-tile pair (2p, 2p+1): both tiles land in
            one two-bank PSUM tile so a single 1024-wide ACT exp covers them."""
            def f():
                stq = ps.tile(
                    [P, 2, TCH], F32, tag="stq", bufs=2, name=f"st_{b}_{j}_{p}"
                )
                s_half(stq, 0, 2 * p)
                s_half(stq, 1, 2 * p + 1)
                pt = work.tile(
                    [P, 2, TCH], BF16, tag="pt", bufs=6, name=f"pt_{b}_{j}_{p}"
                )
                pts[2 * p] = pt[:, 0, :]
                pts[2 * p + 1] = pt[:, 1, :]
                nc.scalar.activation(
                    pt, stq, mybir.ActivationFunctionType.Exp, scale=scale
                )
                for i in (2 * p, 2 * p + 1):
                    if i >= ntt * j:  # diagonal tile: apply causal mask
                        # keep where (j*TCH + tt) - (i*P + ss) >= 0
                        nc.gpsimd.affine_select(
                            out=pts[i],
                            in_=pts[i],
                            compare_op=mybir.AluOpType.is_ge,
                            fill=0.0,
                            base=j * TCH - i * P,
                            channel_multiplier=-1,
                            pattern=[[1, TCH]],
                        )

            return f

        def av_pair(p):
            """AV for the s-tile pair (2p, 2p+1): v and the softmax-denominator
            ones column ride in one M=65 matmul (vaug column 64 is 1.0)."""
            def f():
                if "ut" not in box:
                    box["ut"] = ps.tile(
                        [P, TCH], F32, tag="acc", bufs=1, name=f"ut_{b}_{j}"
                    )
                for i in (2 * p, 2 * p + 1):
                    nc.tensor.matmul(
                        box["ut"][0 : HD + 1, :],
                        vaug[b][:, i, :],
                        pts.pop(i),
                        start=(i == 0),
                        stop=(i == ni - 1),
                        skip_group_check=True,
                    )

            return f

        # pair p's AV rides with pair p+2's scores, giving each 1024-wide exp
        # two pair-quanta of PE work (plus woven A-ops) to complete
        npair = ni // 2
        for p in range(npair):
            if p < 2:
                opsl.append(s_exp(p))
            else:
                sf, af = s_exp(p), av_pair(p - 2)

                def f(sf=sf, af=af):
                    sf()
                    af()

                opsl.append(f)
        for p in range(max(npair - 2, 0), npair):
            opsl.append(av_pair(p))

        def ut_out():
            box["utsb"] = work.tile(
                [P, TCH], F32, tag="utsb", bufs=2, name=f"us_{b}_{j}"
            )
            nc.vector.tensor_copy(box["utsb"][0 : HD + 1, :], box["ut"][0 : HD + 1, :])

        opsl.append(ut_out)

        def ut_tp():
            otp = ps.tile([P, ntt, HD + 1], F32, tag="tp", bufs=3, name=f"ot_{b}_{j}")
            box["otp"] = otp
            for tt in range(ntt):
                nc.tensor.transpose(
                    otp[:, tt, :],
                    box["utsb"][0 : HD + 1, tt * P : (tt + 1) * P],
                    identf[0 : HD + 1, 0 : HD + 1],
                )

        opsl.append(ut_tp)

        def store():
            otp = box["otp"]
            rec = work.tile([P, ntt], F32, tag="rec", bufs=2, name=f"rec_{b}_{j}")
            nc.vector.reciprocal(rec, otp[:, :, HD])
            osb = io_pool.tile(
                [P, ntt, HD], F32, tag="osb", bufs=2, name=f"osb_{b}_{j}"
            )
            # softmax divide: scale-copies alternate ACT/DVE, one DMA per chunk
            for tt in range(ntt):
                if tt % 2 == 0:
                    nc.scalar.mul(osb[:, tt, :], otp[:, tt, 0:HD], rec[:, tt : tt + 1])
                else:
                    nc.vector.tensor_scalar_mul(
                        osb[:, tt, :], otp[:, tt, 0:HD], rec[:, tt : tt + 1]
                    )
            jt0 = j * TCH
            nc.sync.dma_start(
                out[b, jt0 : jt0 + TCH, :].rearrange("(tt p) h -> p tt h", p=P), osb
            )

        opsl.append(store)
        return opsl

    def weave(a_ops, b_ops):
        if not b_ops:
            for op in a_ops:
                op()
            return
        na, nb = len(a_ops), len(b_ops)
        ai = 0
        for k, bop in enumerate(b_ops):
            bop()
            upto = (k + 1) * na // nb
            while ai < upto:
                a_ops[ai]()
                ai += 1

    def spread(base, extra):
        """Distribute `extra` closures evenly through `base`."""
        if not base:
            return list(extra)
        merged = []
        ne, nb = len(extra), len(base)
        ei = 0
        for k, op in enumerate(base):
            merged.append(op)
            upto = (k + 1) * ne // nb
            while ei < upto:
                merged.append(extra[ei])
                ei += 1
        return merged

    # Schedule: slot k runs projections for chunk k woven into attention for
    # chunk k-1; x loads for chunk k+1 are spread through slot k.
    chunks = [(b, j) for j in range(nj) for b in range(b_l)]
    n = len(chunks)
    for op in dma_chunk_ops(*chunks[0], split=True):
        op()
    if n > 1:
        for op in dma_chunk_ops(*chunks[1]):
            op()
    for k in range(n + 1):
        a_ops = a_chunk_ops(*chunks[k]) if k < n else []
        if k + 2 < n:
            a_ops = spread(a_ops, dma_chunk_ops(*chunks[k + 2]))
        b_ops = b_chunk_ops(*chunks[k - 1]) if k >= 1 else []
        weave(a_ops, b_ops)


def build_nc(b_l=BL, t_dim=T, c_dim=C):
    nc = bacc.Bacc("TRN2", target_bir_lowering=False, debug=False)
    x = nc.dram_tensor("x", [b_l, t_dim, c_dim], F32, kind="ExternalInput").ap()
    wk = nc.dram_tensor("Wk", [c_dim, HD], F32, kind="ExternalInput").ap()
    wq = nc.dram_tensor("Wq", [c_dim, HD], F32, kind="ExternalInput").ap()
    wv = nc.dram_tensor("Wv", [c_dim, HD], F32, kind="ExternalInput").ap()
    out = nc.dram_tensor("out", [b_l, t_dim, HD], F32, kind="ExternalOutput").ap()
    from contextlib import ExitStack

    with tile.TileContext(nc) as tc, ExitStack() as ctx:
        build_attention(ctx, tc, out, x, wk, wq, wv, b_l, t_dim, c_dim)
    nc.compile()
    return nc


_NC_CACHE = {}


def _get_nc():
    if "nc" not in _NC_CACHE:
        _NC_CACHE["nc"] = build_nc()
    return _NC_CACHE["nc"]


def kernel(x, Wk, Wq, Wv, _trace=False, _tmpdir=None):
    x = np.ascontiguousarray(np.asarray(x, dtype=np.float32))
    Wk = np.ascontiguousarray(np.asarray(Wk, dtype=np.float32))
    Wq = np.ascontiguousarray(np.asarray(Wq, dtype=np.float32))
    Wv = np.ascontiguousarray(np.asarray(Wv, dtype=np.float32))
    nc = _get_nc()
    in_maps = [
        {"x": x[c * BL : (c + 1) * BL], "Wk": Wk, "Wq": Wq, "Wv": Wv}
        for c in range(N_CORES)
    ]
    res = run_bass_kernel_spmd(
        nc, in_maps, core_ids=list(range(N_CORES)), trace=_trace, tmpdir=_tmpdir
    )
    out = np.concatenate([res.results[c]["out"] for c in range(N_CORES)], axis=0)
    if _trace:
        kernel.last_exec_time_ns = res.exec_time_ns
        kernel.last_results = res
    return out


# revision 34
# speedup vs baseline: 1.1864x; 1.1864x over previous
"""Trainium2 Bass kernel: single-head causal attention (B=16, T=2048, C=1024, HD=64).

Data-parallel over batch across 8 NeuronCores (2 batches/core), weights
replicated. Each core computes, per batch:
    q = x @ Wq, k = x @ Wk, v = x @ Wv            (via transposed layouts)
    scores[t, s] = k[t] . q[s] / sqrt(C)          (computed transposed: St[s, t])
    causal mask (keep s <= t), softmax over s, out[t] = sum_s w[t, s] v[s]

Pipeline layout (all matmuls bf16, PSUM fp32, softmax division fp32):
  - x tiles [128t, 1024c] are cast to bf16 (Pool/DVE), PE-transposed in bf16
    (full-rate loads + streams), and copied out of PSUM at DVE 2x-mode rate.
  - qT/kT come from one stacked [Wq|Wk] projection. Two swapped-stack SBUF
    tensors (qk = q@0:64/k@64:128, kq = k@0:64/q@64:128) let the K=64 score
    matmuls run two-at-a-time in separate PE row groups while needing only
    two cheap bf16 4x SBUF dup-copies per chunk.
  - v is projected with even/odd c-chunks in separate PE column groups,
    cast to bf16, PE-transposed; the two column-group halves are summed by
    a single small DVE add into V_aug [128s, 65] whose column 64 is 1.0 -
    the AV matmul then computes the numerator (rows 0:64) and the softmax
    denominator (row 64) at once.
  - St tile [128s, 512t] = matmul(lhsT=q slice, rhs=k chunk); ACT does
    exp(St/32) straight out of PSUM (no max-subtraction: scores have std
    1/4); gpsimd affine_select masks diagonal tiles; AV accumulates
    ut [65, 512t] in PSUM.
  - ut rows 0:65 are PE-transposed back to [t, 65]; one strided DVE
    reciprocal per chunk + ACT scale-copies produce the output tiles.
  - Emission interleaves projection work (PE/DVE/Pool-heavy) into the
    attention chunks (ACT-heavy) at single-matmul granularity so every
    engine stays dense and the PE holds its top p-state.
"""

import numpy as np

import concourse.bass as bass
import concourse.tile as tile
from concourse import bacc, mybir
from concourse.bass_utils import run_bass_kernel_spmd
from concourse.masks import make_identity

F32 = mybir.dt.float32
BF16 = mybir.dt.bfloat16

B, T, C, HD = 16, 2048, 1024, 64
N_CORES = 8
BL = B // N_CORES  # batches per core

P = 128
TCH = 512  # t-chunk (PSUM bank width in fp32)


def build_attention(ctx, tc, out, x, wk, wq, wv, b_l, t_dim, c_dim):
    nc = tc.nc
    ncc = c_dim // P        # c chunks (contraction)
    nj = t_dim // TCH       # t chunks
    ntt = TCH // P          # t subtiles per chunk
    nst = t_dim // P        # s tiles
    scale = 1.0 / float(np.sqrt(np.float32(c_dim)))

    const_pool = ctx.enter_context(tc.tile_pool(name="const", bufs=1))
    io_pool = ctx.enter_context(tc.tile_pool(name="io", bufs=1))
    big_pool = ctx.enter_context(tc.tile_pool(name="big", bufs=1))
    per_b = ctx.enter_context(tc.tile_pool(name="per_b", bufs=1))
    work = ctx.enter_context(tc.tile_pool(name="work", bufs=1))
    ps = ctx.enter_context(tc.tile_pool(name="ps", bufs=1, space="PSUM"))

    # Identities arrive by DMA (gpsimd is busy generating x-load descriptors
    # at startup and the first PE transposes need identb immediately)
    eye_bf = (np.eye(P) * 0x3F80).astype(np.uint16)  # bf16 bits of 1.0
    identb_dram = nc.inline_tensor(eye_bf, name="identb_const")
    identb = const_pool.tile([P, P], BF16, name="identb")
    nc.sync.dma_start(identb, identb_dram.ap().bitcast(BF16))
    identf_dram = nc.inline_tensor(np.eye(P, dtype=np.float32), name="identf_const")
    identf = const_pool.tile([P, P], F32, name="identf")
    nc.sync.dma_start(identf, identf_dram.ap())

    # Stationary weights, cast to bf16: [c_inner=128, c_chunk, heads].
    wqk_f = const_pool.tile([P, ncc, 2 * HD], F32, name="wqk_f")
    nc.sync.dma_start(wqk_f[:, :, 0:HD], wq.rearrange("(o p) h -> p o h", p=P))
    nc.sync.dma_start(wqk_f[:, :, HD : 2 * HD], wk.rearrange("(o p) h -> p o h", p=P))
    wv_f = const_pool.tile([P, ncc, HD], F32, name="wv_f")
    nc.sync.dma_start(wv_f[:], wv.rearrange("(o p) h -> p o h", p=P))
    wqk_sb = const_pool.tile([P, ncc, 2 * HD], BF16, name="wqk_sb")
    nc.vector.tensor_copy(wqk_sb[:], wqk_f[:])
    wv_sb = const_pool.tile([P, ncc, HD], BF16, name="wv_sb")
    nc.vector.tensor_copy(wv_sb[:], wv_f[:])

    # qk holds the projection output (q rows 0:64, k rows 64:128); kq holds a
    # duplicate of q on rows 64:128 so score matmuls find both operands on the
    # same partitions. vaug holds V^T per s-tile with column 64 set to 1.0.
    qk = {}
    kq = {}
    vaug = {}
    for b in range(b_l):
        qk[b] = per_b.tile([P, t_dim], BF16, name=f"qk_{b}", tag="qk", bufs=2)
        kq[b] = per_b.tile([P, t_dim], BF16, name=f"kq_{b}", tag="kq", bufs=2)
        vaug[b] = per_b.tile(
            [P, nst, HD + 1], BF16, name=f"vaug_{b}", tag="vaug", bufs=2
        )
        nc.vector.memset(vaug[b][:, :, HD], 1.0)

    xins = {}

    def dma_chunk_ops(b, j, split=False):
        """x loads for chunk (b, j) as closures: converting DMAs (fp32 HBM ->
        bf16 SBUF) initiated by gpsimd (only the software DGE can cast).
        split=True halves each load along c so the first transposes (which
        need the low c-chunks of every tile) can start sooner."""
        tiles = []
        opsl = []
        for tt in range(ntt):
            xin = io_pool.tile(
                [P, c_dim], BF16, tag="xin", bufs=12, name=f"xin_{b}_{j}_{tt}"
            )
            tiles.append(xin)
        xins[(b, j)] = tiles

        def load(tt, lo, hi):
            def f():
                t0 = j * TCH + tt * P
                nc.gpsimd.dma_start(tiles[tt][:, lo:hi], x[b, t0 : t0 + P, lo:hi])

            return f

        if split and c_dim >= 2 * P:
            h = c_dim // 2
            for tt in range(ntt):
                opsl.append(load(tt, 0, h))
            for tt in range(ntt):
                opsl.append(load(tt, h, c_dim))
        else:
            for tt in range(ntt):
                opsl.append(load(tt, 0, c_dim))
        return opsl

    def a_chunk_ops(b, j):
        """Projection chunk: transpose, project q/k/v. Returns closures."""
        opsl = []
        xinb = xins.pop((b, j))

        xT = big_pool.tile(
            [P, ncc, TCH], BF16, tag="xT", bufs=2, name=f"xT_{b}_{j}"
        )
        tps = {}

        def tp_fill(g):
            def f():
                tp = ps.tile(
                    [P, 2, TCH], BF16, tag="tp", bufs=3, name=f"tp_{b}_{j}_{g}"
                )
                tps[g] = tp
                for ccp in range(2):
                    cc = 2 * g + ccp
                    for tt in range(ntt):
                        nc.tensor.transpose(
                            tp[:, ccp, tt * P : (tt + 1) * P],
                            xinb[tt][:, cc * P : (cc + 1) * P],
                            identb,
                        )

            return f

        def tp_copy(g):
            def f():
                nc.vector.tensor_copy(xT[:, 2 * g : 2 * g + 2, :], tps.pop(g))

            return f

        for g in range(ncc // 2):
            opsl.append(tp_fill(g))
            opsl.append(tp_copy(g))

        qkps_box = {}

        def qk_mm(c0, c1):
            def f():
                if "t" not in qkps_box:
                    qkps_box["t"] = ps.tile(
                        [P, TCH], F32, tag="tp", bufs=3, name=f"qkps_{b}_{j}"
                    )
                for cc in range(c0, c1):
                    nc.tensor.matmul(
                        qkps_box["t"],
                        wqk_sb[:, cc, :],
                        xT[:, cc, :],
                        start=(cc == 0),
                        stop=(cc == ncc - 1),
                        skip_group_check=True,
                    )

            return f

        opsl.append(qk_mm(0, ncc // 2))
        opsl.append(qk_mm(ncc // 2, ncc))

        jt = slice(j * TCH, (j + 1) * TCH)

        def qk_out():
            qkps = qkps_box["t"]
            nc.vector.tensor_copy(qk[b][:, jt], qkps)  # cast fp32 -> bf16
            # duplicate q onto partitions 64:128 (bf16 sbuf->sbuf, fast mode)
            nc.vector.tensor_copy(kq[b][HD:P, jt], qk[b][0:HD, jt])

        opsl.append(qk_out)

        vps_box = {}

        def v_mm(c0, c1):
            def f():
                if "t" not in vps_box:
                    vps_box["t"] = ps.tile(
                        [P, TCH], F32, tag="tp", bufs=3, name=f"vps_{b}_{j}"
                    )
                vps = vps_box["t"]
                for cc in range(c0, c1):
                    h = cc % 2
                    nc.tensor.matmul(
                        vps[h * HD : (h + 1) * HD, :],
                        wv_sb[:, cc, :],
                        xT[:, cc, :],
                        start=(cc == h),
                        stop=(cc == ncc - 2 + h),
                        tile_position=(0, h * HD),
                        skip_group_check=True,
                    )

            return f

        opsl.append(v_mm(0, ncc // 2))
        opsl.append(v_mm(ncc // 2, ncc))

        vsb_box = {}

        def v_out():
            vsb_box["t"] = work.tile([P, TCH], BF16, tag="vsb", bufs=2, name=f"vs_{b}_{j}")
            nc.vector.tensor_copy(vsb_box["t"], vps_box["t"])  # cast

        opsl.append(v_out)

        vtp_box = {}

        def v_tp():
            vtp = ps.tile([P, 2, TCH], BF16, tag="tp", bufs=3, name=f"vtp_{b}_{j}")
            vtp_box["t"] = vtp
            vsb = vsb_box["t"]
            for tt in range(ntt):
                nc.tensor.transpose(
                    vtp[:, 0, tt * P : (tt + 1) * P],
                    vsb[:, tt * P : (tt + 1) * P],
                    identb,
                )

        opsl.append(v_tp)

        def v_aug():
            vtsb = work.tile([P, ntt, P], BF16, tag="vtsb", bufs=2, name=f"vt_{b}_{j}")
            nc.vector.tensor_copy(vtsb, vtp_box["t"][:, 0, :])
            # sum the two column-group halves of every subtile in one op
            nc.vector.tensor_add(
                vaug[b][:, j * ntt : (j + 1) * ntt, 0:HD],
                vtsb[:, :, 0:HD],
                vtsb[:, :, HD:P],
            )

        opsl.append(v_aug)
        return opsl

    def b_chunk_ops(b, j):
        """Attention chunk: scores, exp, mask, AV, softmax-divide, store."""
        opsl = []
        ni = ntt * j + ntt  # s-tiles with any valid (s <= t) entry
        jt = slice(j * TCH, (j + 1) * TCH)
        pts = {}
        box = {}

        def s_half(stq, hh, i):
            """Scores for s-tile i into stq half hh. Single K=64 matmul on PE
            rows 64:128 (cost is column-limited; K/M don't matter)."""
            nc.tensor.matmul(
                stq[:, hh, :],
                kq[b][HD:P, i * P : (i + 1) * P],
                qk[b][HD:P, jt],
                start=True,
                stop=True,
                tile_position=(HD, 0),
                skip_group_check=True,
            )

        def s_exp(p):
            """Scores + exp for the s-tile pair (2p, 2p+1): both tiles land in
            one two-bank PSUM tile so a single 1024-wide ACT exp covers them."""
            def f():
                stq = ps.tile(
                    [P, 2, TCH], F32, tag="stq", bufs=2, name=f"st_{b}_{j}_{p}"
                )
                s_half(stq, 0, 2 * p)
                s_half(stq, 1, 2 * p + 1)
                pt = work.tile(
                    [P, 2, TCH], BF16, tag="pt", bufs=6, name=f"pt_{b}_{j}_{p}"
                )
                pts[2 * p] = pt[:, 0, :]
                pts[2 * p + 1] = pt[:, 1, :]
                nc.scalar.activation(
                    pt, stq, mybir.ActivationFunctionType.Exp, scale=scale
                )
                for i in (2 * p, 2 * p + 1):
                    if i >= ntt * j:  # diagonal tile: apply causal mask
                        # keep where (j*TCH + tt) - (i*P + ss) >= 0
                        nc.gpsimd.affine_select(
                            out=pts[i],
                            in_=pts[i],
                            compare_op=mybir.AluOpType.is_ge,
                            fill=0.0,
                            base=j * TCH - i * P,
                            channel_multiplier=-1,
                            pattern=[[1, TCH]],
                        )

            return f

        def av_pair(p):
            """AV for the s-tile pair (2p, 2p+1): v and the softmax-denominator
            ones column ride in one M=65 matmul (vaug column 64 is 1.0)."""
            def f():
                if "ut" not in box:
                    box["ut"] = ps.tile(
                        [P, TCH], F32, tag="acc", bufs=1, name=f"ut_{b}_{j}"
                    )
                for i in (2 * p, 2 * p + 1):
                    nc.tensor.matmul(
                        box["ut"][0 : HD + 1, :],
                        vaug[b][:, i, :],
                        pts.pop(i),
                        start=(i == 0),
                        stop=(i == ni - 1),
                        skip_group_check=True,
                    )

            return f

        # pair p's AV rides with pair p+2's scores, giving each 1024-wide exp
        # two pair-quanta of PE work (plus woven A-ops) to complete
        npair = ni // 2
        for p in range(npair):
            if p < 2:
                opsl.append(s_exp(p))
            else:
                sf, af = s_exp(p), av_pair(p - 2)

                def f(sf=sf, af=af):
                    sf()
                    af()

                opsl.append(f)
        for p in range(max(npair - 2, 0), npair):
            opsl.append(av_pair(p))

        def ut_out():
            box["utsb"] = work.tile(
                [P, TCH], F32, tag="utsb", bufs=2, name=f"us_{b}_{j}"
            )
            nc.vector.tensor_copy(box["utsb"][0 : HD + 1, :], box["ut"][0 : HD + 1, :])

        opsl.append(ut_out)

        def ut_tp():
            otp = ps.tile([P, ntt, HD + 1], F32, tag="tp", bufs=3, name=f"ot_{b}_{j}")
            box["otp"] = otp
            for tt in range(ntt):
                nc.tensor.transpose(
                    otp[:, tt, :],
                    box["utsb"][0 : HD + 1, tt * P : (tt + 1) * P],
                    identf[0 : HD + 1, 0 : HD + 1],
                )

        opsl.append(ut_tp)

        def store():
            otp = box["otp"]
            rec = work.tile([P, ntt], F32, tag="rec", bufs=2, name=f"rec_{b}_{j}")
            nc.vector.reciprocal(rec, otp[:, :, HD])
            osb = io_pool.tile(
                [P, ntt, HD], F32, tag="osb", bufs=2, name=f"osb_{b}_{j}"
            )
            # softmax divide: scale-copies alternate ACT/DVE, one DMA per chunk
            for tt in range(ntt):
                nc.vector.tensor_scalar_mul(
                    osb[:, tt, :], otp[:, tt, 0:HD], rec[:, tt : tt + 1]
                )
            jt0 = j * TCH
            nc.sync.dma_start(
                out[b, jt0 : jt0 + TCH, :].rearrange("(tt p) h -> p tt h", p=P), osb
            )

        opsl.append(store)
        return opsl

    def weave(a_ops, b_ops):
        if not b_ops:
            for op in a_ops:
                op()
            return
        na, nb = len(a_ops), len(b_ops)
        ai = 0
        for k, bop in enumerate(b_ops):
            bop()
            upto = (k + 1) * na // nb
            while ai < upto:
                a_ops[ai]()
                ai += 1

    def spread(base, extra):
        """Distribute `extra` closures evenly through `base`."""
        if not base:
            return list(extra)
        merged = []
        ne, nb = len(extra), len(base)
        ei = 0
        for k, op in enumerate(base):
            merged.append(op)
            upto = (k + 1) * ne // nb
            while ei < upto:
                merged.append(extra[ei])
                ei += 1
        return merged

    # Schedule: slot k runs projections for chunk k woven into attention for
    # chunk k-1; x loads for chunk k+1 are spread through slot k.
    chunks = [(b, j) for j in range(nj) for b in range(b_l)]
    n = len(chunks)
    for op in dma_chunk_ops(*chunks[0], split=True):
        op()
    if n > 1:
        for op in dma_chunk_ops(*chunks[1]):
            op()
    for k in range(n + 1):
        a_ops = a_chunk_ops(*chunks[k]) if k < n else []
        if k + 2 < n:
            a_ops = spread(a_ops, dma_chunk_ops(*chunks[k + 2]))
        b_ops = b_chunk_ops(*chunks[k - 1]) if k >= 1 else []
        weave(a_ops, b_ops)


def build_nc(b_l=BL, t_dim=T, c_dim=C):
    nc = bacc.Bacc("TRN2", target_bir_lowering=False, debug=False)
    x = nc.dram_tensor("x", [b_l, t_dim, c_dim], F32, kind="ExternalInput").ap()
    wk = nc.dram_tensor("Wk", [c_dim, HD], F32, kind="ExternalInput").ap()
    wq = nc.dram_tensor("Wq", [c_dim, HD], F32, kind="ExternalInput").ap()
    wv = nc.dram_tensor("Wv", [c_dim, HD], F32, kind="ExternalInput").ap()
    out = nc.dram_tensor("out", [b_l, t_dim, HD], F32, kind="ExternalOutput").ap()
    from contextlib import ExitStack

    with tile.TileContext(nc) as tc, ExitStack() as ctx:
        build_attention(ctx, tc, out, x, wk, wq, wv, b_l, t_dim, c_dim)
    nc.compile()
    return nc


_NC_CACHE = {}


def _get_nc():
    if "nc" not in _NC_CACHE:
        _NC_CACHE["nc"] = build_nc()
    return _NC_CACHE["nc"]


def kernel(x, Wk, Wq, Wv, _trace=False, _tmpdir=None):
    x = np.ascontiguousarray(np.asarray(x, dtype=np.float32))
    Wk = np.ascontiguousarray(np.asarray(Wk, dtype=np.float32))
    Wq = np.ascontiguousarray(np.asarray(Wq, dtype=np.float32))
    Wv = np.ascontiguousarray(np.asarray(Wv, dtype=np.float32))
    nc = _get_nc()
    in_maps = [
        {"x": x[c * BL : (c + 1) * BL], "Wk": Wk, "Wq": Wq, "Wv": Wv}
        for c in range(N_CORES)
    ]
    res = run_bass_kernel_spmd(
        nc, in_maps, core_ids=list(range(N_CORES)), trace=_trace, tmpdir=_tmpdir
    )
    out = np.concatenate([res.results[c]["out"] for c in range(N_CORES)], axis=0)
    if _trace:
        kernel.last_exec_time_ns = res.exec_time_ns
        kernel.last_results = res
    return out
